# revision 1
# baseline (speedup 1.0000x reference)

"""Causal attention (no head split) on 8 trn2 NeuronCores.

Reference computation (per batch b):
    q = x @ Wq^T ; k = x @ Wk^T ; v = x @ Wv^T          (nn.Linear convention)
    wei = softmax(mask(q @ k^T / sqrt(C)))               (causal)
    out = wei @ v

Algebraic restructuring (K and V are never materialized):
    S   = q k^T = x (Wq^T Wk) x^T = x M x^T     with M precomputed on host
    out = wei v = (wei x) Wv^T, i.e. O^T = Wv (x^T wei^T) = Wv H
so the device only computes:
    G^T = M^T xq^T                  (one projection of this core's queries)
    S^T[s,t] = x^T(lhsT) G^T(rhs)   (contract over C)
    P^T = exp(S^T / 32) * mask ; rowsum[t] += ones^T P^T
    H[c,t] += x(lhsT) P^T(rhs)      (contract over s, accumulated in SBUF)
    O^T = Wv^T-projection of H      (once per finished query strip)
Final softmax normalization (divide by rowsum) happens on the host.

Sharding: 2 cores per batch (B=4). Queries split into eight 256-row strips;
role A takes strips {0,2,4,6} (rows [512j,512j+256)), role B {1,3,5,7}.
Every core runs the IDENTICAL instruction stream (single SPMD NEFF); role
differences are carried entirely by input data (query columns + mask tiles).
Everything is fp32r (e8m11) on the PE: bf16 speed, ~8x bf16 precision.
"""
import os
import numpy as np

import concourse.bass as bass
from concourse import bacc
import concourse.mybir as mybir
from concourse.tile import TileContext
from concourse import bass_utils

B, T, C = 4, 2048, 1024
P = 128
CS = C // P          # 8 contraction subtiles
NCH = T // 256       # 8 kv chunks of 256
QS = 4               # query strips per core
SW = 256             # strip width
SCALE = 1.0 / np.sqrt(C)  # 1/32

F32R = mybir.dt.float32r
F32 = mybir.dt.float32


def round_fp32r(x: np.ndarray) -> np.ndarray:
    """Round fp32 to fp32r (e8m11): round-to-nearest-even to 11 mantissa bits."""
    x = np.ascontiguousarray(x, dtype=np.float32)
    bits = x.view(np.uint32)
    lsb = (bits >> 12) & 1
    out = (bits + 0x7FF + lsb) & np.uint32(0xFFFFF000)
    return out.view(np.float32)


def build():
    nc = bacc.Bacc(trn_type="TRN2", name="causal_attn")
    xT = nc.dram_tensor("xT", [C, T], F32R, kind="ExternalInput")    # x^T (batch)
    xn = nc.dram_tensor("xn", [T, C], F32R, kind="ExternalInput")    # x natural
    xqT = nc.dram_tensor("xqT", [C, QS * SW], F32R, kind="ExternalInput")
    wm = nc.dram_tensor("wm", [C, C], F32R, kind="ExternalInput")    # M = Wq^T Wk
    wvT = nc.dram_tensor("wvT", [C, C], F32R, kind="ExternalInput")  # Wv^T [c,d]
    masks = nc.dram_tensor("masks", [P, 4, SW], F32R, kind="ExternalInput")
    ones = nc.dram_tensor("ones", [P, 1], F32R, kind="ExternalInput")
    outT = nc.dram_tensor("outT", [C, QS * SW], F32, kind="ExternalOutput")
    rows = nc.dram_tensor("rows", [1, QS * SW], F32, kind="ExternalOutput")

    xT_r = xT.rearrange("(cs p) t -> p cs t", p=P)
    xn_r = xn.rearrange("(ch ss p) c -> p ch ss c", p=P, ss=2)
    xqT_r = xqT.rearrange("(cs p) t -> p cs t", p=P)
    wm_r = wm.rearrange("(cs p) d -> p cs d", p=P)
    wvT_r = wvT.rearrange("(cs p) d -> p cs d", p=P)
    outT_r = outT.rearrange("(ds p) t -> p ds t", p=P)
    rows_r = rows.rearrange("p (a b) -> p a b", a=QS)

    with TileContext(nc) as tc:
        with tc.tile_pool(name="keep", bufs=1) as keep, \
             tc.tile_pool(name="wpool", bufs=2) as wpool, \
             tc.tile_pool(name="stream", bufs=3) as stream, \
             tc.tile_pool(name="hrpool", bufs=1) as hrpool, \
             tc.tile_pool(name="ppool", bufs=3) as ppool, \
             tc.tile_pool(name="psA", bufs=2, space="PSUM") as psA, \
             tc.tile_pool(name="psS", bufs=3, space="PSUM") as psS, \
             tc.tile_pool(name="psO", bufs=2, space="PSUM") as psO, \
             tc.tile_pool(name="psR", bufs=1, space="PSUM") as psR:

            gT = keep.tile([P, CS, QS * SW], F32R, tag="gT")   # G^T  32KB/part
            hh = keep.tile([P, CS, QS * SW], F32, tag="hh")    # H    32KB/part
            msk = keep.tile([P, 4, SW], F32R, tag="msk")
            ones_t = keep.tile([P, 1], F32R, tag="ones")
            rowsum = keep.tile([1, QS, SW], F32, tag="rowsum")
            # ---- Phase G: G^T = M^T xq^T for the 4 query strips ----
            # first group's weight slice goes out first (longest pole), split
            # in halves across two queues; then the strip-0 queries.
            wq = wpool.tile([P, CS, C], F32R, tag="w")
            for h in range(2):
                nc.sync.dma_start(wq[:, 4 * h:4 * h + 4, 0:P],
                                  wm_r[:, 4 * h:4 * h + 4, 0:P])
            xq0 = stream.tile([P, CS, SW], F32R, tag="xt")
            for h in range(4):
                nc.sync.dma_start(
                    xq0[:, 2 * h:2 * h + 2],
                    xqT_r[:, 2 * h:2 * h + 2, 0:SW])
            for ds in range(1, CS):
                nc.sync.dma_start(wq[:, :, ds * P:(ds + 1) * P],
                                  wm_r[:, :, ds * P:(ds + 1) * P])
            nc.sync.dma_start(msk[:], masks[:])
            nc.sync.dma_start(ones_t[:], ones[:])
            for j in range(QS):
                if j == 0:
                    xq = xq0
                else:
                    xq = stream.tile([P, CS, SW], F32R, tag="xt")
                    for h in range(4):
                        nc.sync.dma_start(
                            xq[:, 2 * h:2 * h + 2],
                            xqT_r[:, 2 * h:2 * h + 2, j * SW:(j + 1) * SW])
                for ds in range(CS):
                    pq = psA.tile([P, SW], F32, tag="prod")
                    for cs in range(CS):
                        nc.tensor.matmul(
                            pq[:], wq[:, cs, ds * P:(ds + 1) * P], xq[:, cs],
                            start=(cs == 0), stop=(cs == CS - 1))
                    nc.scalar.copy(gT[:, ds, j * SW:(j + 1) * SW], pq[:])

            # Wv^T for the final output projections (second w slot)
            wv = wpool.tile([P, CS, C], F32R, tag="w")
            for dh in range(2):
                nc.sync.dma_start(wv[:, :, dh * 512:(dh + 1) * 512],
                                  wvT_r[:, :, dh * 512:(dh + 1) * 512])

            # ---- Chunk loop: stream x^T / x for chunk c, attend all strips.
            # Order ends at chunks 4,5 so strips 2 AND 3 both complete near the
            # end and their Wv-projections interleave (fills the tail chain).
            CHUNK_ORDER = [0, 1, 2, 3, 6, 7, 4, 5]
            LAST_VISIT = {j: max(range(NCH), key=lambda p: (CHUNK_ORDER[p] <= 2 * j + 1, p))
                          for j in range(QS)}
            for pos in range(NCH):
                c = CHUNK_ORDER[pos]
                xt = stream.tile([P, CS, 256], F32R, tag="xt")
                for h in range(4):
                    nc.sync.dma_start(
                        xt[:, 2 * h:2 * h + 2],
                        xT_r[:, 2 * h:2 * h + 2, c * 256:(c + 1) * 256])
                xna = stream.tile([P, 2, C], F32R, tag="xn")
                for ss in range(2):
                    nc.sync.dma_start(xna[:, ss], xn_r[:, c, ss])

                # strips that attend to chunk c: 2j+1 >= c
                for j in range(QS):
                    if 2 * j + 1 < c:
                        continue
                    tsl = slice(j * SW, (j + 1) * SW)

                    st = psS.tile([P, 2, SW], F32, tag="st")
                    for ss in range(2):
                        for cs in range(CS):
                            nc.tensor.matmul(
                                st[:, ss], xt[:, cs, ss * P:(ss + 1) * P],
                                gT[:, cs, tsl],
                                start=(cs == 0), stop=(cs == CS - 1))

                    pT = ppool.tile([P, 2, SW], F32R, tag="pT")
                    nc.scalar.activation(
                        pT[:], st[:],
                        mybir.ActivationFunctionType.Exp, scale=float(SCALE))

                    midx = None
                    if c == 2 * j:
                        midx = 0
                    elif c == 2 * j + 1:
                        midx = 1
                    if midx is not None:
                        nc.vector.tensor_mul(
                            pT[:], pT[:], msk[:, midx * 2:midx * 2 + 2])

                    rw = psR.tile([1, SW], F32, tag="rw")
                    for ss in range(2):
                        nc.tensor.matmul(
                            rw[:], ones_t[:], pT[:, ss],
                            start=(ss == 0), stop=(ss == 1))
                    if c == 0:
                        nc.vector.tensor_copy(rowsum[:, j], rw[:])
                    else:
                        nc.vector.tensor_add(rowsum[:, j], rowsum[:, j], rw[:])

                    # H[c,t] += x(lhsT) @ P^T, c-subtiles in quarters
                    for q4 in range(4):
                        po = psO.tile([P, 2, SW], F32, tag="po")
                        for i in range(2):
                            cs4 = 2 * q4 + i
                            for ss in range(2):
                                nc.tensor.matmul(
                                    po[:, i], xna[:, ss, cs4 * P:(cs4 + 1) * P],
                                    pT[:, ss],
                                    start=(ss == 0), stop=(ss == 1))
                        hsl = hh[:, 2 * q4:2 * q4 + 2, tsl]
                        if c == 0:
                            nc.vector.tensor_copy(hsl, po[:])
                        else:
                            nc.vector.tensor_add(hsl, hsl, po[:])

                    # strip complete after its last chunk: project by Wv^T.
                    # hr cast per c-half so the projection's early contraction
                    # steps overlap the tail of H accumulation.
                    if pos == LAST_VISIT[j]:
                        hr = hrpool.tile([P, CS, SW], F32R, tag="hr")
                        for q2 in range(2):
                            nc.scalar.copy(hr[:, 4 * q2:4 * q2 + 4],
                                           hh[:, 4 * q2:4 * q2 + 4, tsl])
                        ost = hrpool.tile([P, CS, SW], F32, tag="ost")
                        for ds in range(CS):
                            pf = psA.tile([P, SW], F32, tag="prod")
                            for cs in range(CS):
                                nc.tensor.matmul(
                                    pf[:], wv[:, cs, ds * P:(ds + 1) * P],
                                    hr[:, cs],
                                    start=(cs == 0), stop=(cs == CS - 1))
                            nc.vector.tensor_copy(ost[:, ds], pf[:])
                            nc.sync.dma_start(outT_r[:, ds, tsl], ost[:, ds])

            nc.sync.dma_start(rows_r[:], rowsum[:])

    nc.compile()
    return nc


_NC = None


def _get_nc():
    global _NC
    if _NC is None:
        _NC = build()
    return _NC


def make_in_maps(x, Wq, Wk, Wv):
    x = np.asarray(x, dtype=np.float32)
    wq64 = np.asarray(Wq, np.float64)
    wk64 = np.asarray(Wk, np.float64)
    wm = round_fp32r((wq64.T @ wk64).astype(np.float32))     # M = Wq^T Wk [c',c]
    wvT = round_fp32r(np.asarray(Wv, np.float32).T)
    ones = np.ones((P, 1), np.float32)

    # mask tiles [p, midx*2+ss, t]: tri = 1 if (ss*128+p) <= t
    s_idx = (np.arange(2)[:, None, None] * P + np.arange(P)[None, :, None])
    tri = (s_idx <= np.arange(SW)[None, None, :]).astype(np.float32)
    tri = np.ascontiguousarray(tri.transpose(1, 0, 2))
    zeros = np.zeros((P, 2, SW), np.float32)
    ones2 = np.ones((P, 2, SW), np.float32)
    mask_A = np.ascontiguousarray(np.concatenate([tri, zeros], axis=1), np.float32)
    mask_B = np.ascontiguousarray(np.concatenate([ones2, tri], axis=1), np.float32)

    xr = [round_fp32r(x[b]) for b in range(B)]
    xT = [np.ascontiguousarray(xr[b].T) for b in range(B)]
    in_maps = []
    for core in range(8):
        b, role = divmod(core, 2)
        cols = np.concatenate(
            [np.arange(512 * j + SW * role, 512 * j + SW * role + SW)
             for j in range(QS)])
        xqT = np.ascontiguousarray(xT[b][:, cols])
        in_maps.append({
            "xT": xT[b],
            "xn": xr[b],
            "xqT": xqT,
            "wm": wm, "wvT": wvT,
            "masks": mask_A if role == 0 else mask_B,
            "ones": ones,
        })
    return in_maps


def assemble(results):
    out = np.empty((B, T, C), np.float32)
    for core in range(8):
        b, role = divmod(core, 2)
        oT = results[core]["outT"]                   # [C, 1024]
        rsum = results[core]["rows"].reshape(QS * SW)
        o = oT.T / rsum[:, None]
        for j in range(QS):
            r0 = 512 * j + SW * role
            out[b, r0:r0 + SW] = o[j * SW:(j + 1) * SW]
    return out


def kernel(x, Wq, Wk, Wv):
    nc = _get_nc()
    in_maps = make_in_maps(x, Wq, Wk, Wv)
    res = bass_utils.run_bass_kernel_spmd(nc, in_maps, core_ids=list(range(8)))
    return assemble(res.results)


def _install_trace_shim():
    """Provide antenv.axon_hooks (absent in this image) so trace=True works."""
    import sys
    import types
    if "antenv.axon_hooks" in sys.modules:
        return
    hook_box = [None]
    mod = types.ModuleType("antenv.axon_hooks")
    mod.set_axon_ntff_profile_hook = lambda h: hook_box.__setitem__(0, h)
    mod.get_axon_ntff_profile_hook = lambda: hook_box[0]
    import antenv
    sys.modules["antenv.axon_hooks"] = mod
    antenv.axon_hooks = mod
    try:
        from trn_agent_boot.trn_boot import _ntff_profile_via_ctypes
        mod.set_axon_ntff_profile_hook(
            _ntff_profile_via_ctypes("/opt/axon/libaxon_pjrt.so"))
    except Exception:
        pass


def run_traced(x, Wq, Wk, Wv):
    """Like kernel() but with NTFF tracing; returns (out, BassKernelResults)."""
    _install_trace_shim()
    nc = _get_nc()
    in_maps = make_in_maps(x, Wq, Wk, Wv)
    res = bass_utils.run_bass_kernel_spmd(
        nc, in_maps, core_ids=list(range(8)), trace=True,
        trace_cores=list(range(8)))
    return assemble(res.results), res



# revision 9
# speedup vs baseline: 1.0219x; 1.0219x over previous

"""Causal attention (no head split) on 8 trn2 NeuronCores — v2.

Math (per batch b), K/V never materialized:
    S   = x M x^T with M = Wq^T Wk precomputed on host
    out = (wei x) Wv^T; device computes H = x^T wei^T then O^T = Wv H.

v2 structure (vs v1):
  * strip-major schedule: 8 query strips of 128 rows per core; strip j
    attends key chunks 0..j (256 keys each). Exact causal tiling for both
    roles (role A takes even 128-row blocks, role B odd) — zero fully
    masked visits.
  * logit path bf16; strips >= FP8_FROM run P/xn/H-cast/Wv in fp8 e4m3
    with DoubleRow matmuls; earlier strips (small softmax rows) stay bf16.
  * H and rowsum accumulate in PSUM across a strip's chunks.
  * causal mask applied additively to S *before* exp (no fp8 inf*0).
  * all inputs pre-tiled on host to [128, X] contiguous layouts; one DMA
    per tensor half, staggered so the G-phase gate (wm, xq) lands first.
"""
import numpy as np
import ml_dtypes

import concourse.bass as bass
from concourse import bacc
import concourse.mybir as mybir
from concourse.tile import TileContext
from concourse import bass_utils

B, T, C = 4, 2048, 1024
P = 128
CS = C // P           # 8 contraction subtiles
NS = 8                # strips per core (128 queries each)
SCALE = 1.0 / np.sqrt(C)   # 1/32
EXP_BIAS = -2.5
FP8_FROM = 2          # strips >= this use fp8 path
H_SCALE = 0.25        # H -> fp8 cast scale
WV_SCALE = 64.0       # Wv fp8 scale
OUT_RESCALE = 1.0 / (H_SCALE * WV_SCALE)
MASK_NEG = -65536.0   # additive mask for non-causal logits

BF16 = mybir.dt.bfloat16
FP8 = mybir.dt.float8e4
F32 = mybir.dt.float32
DR = mybir.MatmulPerfMode.DoubleRow

NP_BF16 = ml_dtypes.bfloat16
NP_FP8 = ml_dtypes.float8_e4m3


def _tile128(a):
    """[(g*128), X] row-major -> [128, g, X] (partition-major pre-tiling)."""
    g = a.shape[0] // P
    return np.ascontiguousarray(a.reshape(g, P, -1).transpose(1, 0, 2))


def build():
    nc = bacc.Bacc(trn_type="TRN2", name="causal_attn_v2")
    xt_d = nc.dram_tensor("xt", [P, CS, T], BF16, kind="ExternalInput")
    xq_d = nc.dram_tensor("xq", [P, CS, NS * P], BF16, kind="ExternalInput")
    wm_d = nc.dram_tensor("wm", [P, CS, C], BF16, kind="ExternalInput")
    xnb_d = nc.dram_tensor("xnb", [P, 4, C], BF16, kind="ExternalInput")
    xn8_d = nc.dram_tensor("xn8", [P, 16, C], FP8, kind="ExternalInput")
    wvb_d = nc.dram_tensor("wvb", [P, CS, C], BF16, kind="ExternalInput")
    wv8_d = nc.dram_tensor("wv8", [P, CS, C], FP8, kind="ExternalInput")
    msk_d = nc.dram_tensor("msk", [P, 2, P], BF16, kind="ExternalInput")
    onesb_d = nc.dram_tensor("onesb", [P, 1], BF16, kind="ExternalInput")
    ones8_d = nc.dram_tensor("ones8", [P, 1], FP8, kind="ExternalInput")
    biasv_d = nc.dram_tensor("biasv", [P, 1], F32, kind="ExternalInput")
    outT_d = nc.dram_tensor("outTb", [P, CS, NS * P], BF16, kind="ExternalOutput")
    rows_d = nc.dram_tensor("rows", [1, NS * P], F32, kind="ExternalOutput")
    rows_r = rows_d.rearrange("p (j t) -> p j t", j=NS)

    with TileContext(nc) as tc:
        with tc.tile_pool(name="keep", bufs=1) as keep, \
             tc.tile_pool(name="hrpool", bufs=2) as hrpool, \
             tc.tile_pool(name="opool", bufs=2) as opool, \
             tc.tile_pool(name="ppool", bufs=10) as ppool, \
             tc.tile_pool(name="psA", bufs=2, space="PSUM") as psA, \
             tc.tile_pool(name="psS", bufs=3, space="PSUM") as psS, \
             tc.tile_pool(name="psH", bufs=1, space="PSUM") as psH, \
             tc.tile_pool(name="psR", bufs=1, space="PSUM") as psR:

            # ---------------- resident tiles ----------------
            xt = keep.tile([P, CS, T], BF16, tag="xt")          # 32KB/part
            xq = keep.tile([P, CS, NS * P], BF16, tag="xq")     # 16KB
            wm = keep.tile([P, CS, C], BF16, tag="wm")          # 16KB
            gT = keep.tile([P, CS, NS * P], BF16, tag="gT")     # 16KB
            xnb_t = keep.tile([P, 4, C], BF16, tag="xnb")       # 8KB
            xn8_t = keep.tile([P, 16, C], FP8, tag="xn8")       # 16KB
            wvb_t = keep.tile([P, CS, C], BF16, tag="wvb")      # 16KB
            wv8_t = keep.tile([P, CS, C], FP8, tag="wv8")       # 8KB
            mt = keep.tile([P, 2, P], BF16, tag="mt")
            ob = keep.tile([P, 1], BF16, tag="ob")
            o8 = keep.tile([P, 1], FP8, tag="o8")
            bv = keep.tile([P, 1], F32, tag="bv")
            rsum = keep.tile([1, NS, P], F32, tag="rsum")

            # ---------------- DMAs (contiguous per partition) -----------
            # sync: the G gate first — wm halves + xq halves, staggered.
            nc.sync.dma_start(wm[:, :, 0:512], wm_d[:, :, 0:512])
            nc.sync.dma_start(xq[:, :, 0:512], xq_d[:, :, 0:512])
            nc.sync.dma_start(wm[:, :, 512:1024], wm_d[:, :, 512:1024])
            nc.sync.dma_start(xq[:, :, 512:1024], xq_d[:, :, 512:1024])
            nc.sync.dma_start(mt[:], msk_d[:])
            nc.sync.dma_start(ob[:], onesb_d[:])
            nc.sync.dma_start(o8[:], ones8_d[:])
            nc.sync.dma_start(bv[:], biasv_d[:])
            # gpsimd: keys/values bulk (needed from ~end of G phase on)
            nc.gpsimd.dma_start(xnb_t[:], xnb_d[:])
            for k in range(4):
                nc.gpsimd.dma_start(xt[:, :, 512 * k:512 * (k + 1)],
                                    xt_d[:, :, 512 * k:512 * (k + 1)])
            nc.gpsimd.dma_start(xn8_t[:, 0:8], xn8_d[:, 0:8])
            nc.gpsimd.dma_start(xn8_t[:, 8:16], xn8_d[:, 8:16])
            # scalar: Wv (first use ~pair-0 projection)
            nc.scalar.dma_start(wvb_t[:], wvb_d[:])
            nc.scalar.dma_start(wv8_t[:], wv8_d[:])

            # ---------------- Phase G: gT = M^T xq^T (bf16) -------------
            for q in range(4):
                qs = slice(256 * q, 256 * (q + 1))
                for ds in range(CS):
                    pq = psA.tile([P, 256], F32, tag="pq")
                    for cs in range(CS):
                        nc.tensor.matmul(
                            pq[:], wm[:, cs, ds * P:(ds + 1) * P], xq[:, cs, qs],
                            start=(cs == 0), stop=(cs == CS - 1))
                    nc.vector.tensor_copy(gT[:, ds, qs], pq[:])

            # ---------------- strip-major attention ---------------------
            hr = None
            for j in range(NS):
                f8 = j >= FP8_FROM
                tsl = slice(j * P, (j + 1) * P)
                po = psH.tile([P, CS, P], F32, tag="po")
                rw = psR.tile([1, P], F32, tag="rw")
                pts = []
                for c in range(j + 1):
                    st = psS.tile([P, 2, P], F32, tag="st")
                    for ss in range(2):
                        for cs in range(CS):
                            nc.tensor.matmul(
                                st[:, ss],
                                xt[:, cs, 256 * c + P * ss:256 * c + P * (ss + 1)],
                                gT[:, cs, tsl],
                                start=(cs == 0), stop=(cs == CS - 1))
                    if c == j:  # additive causal mask on the diagonal chunk
                        nc.vector.tensor_add(st[:], st[:], mt[:])
                    pT = ppool.tile([P, 2, P], FP8 if f8 else BF16, tag="pT")
                    nc.scalar.activation(
                        pT[:], st[:], mybir.ActivationFunctionType.Exp,
                        scale=float(SCALE), bias=bv[:])
                    first, last = (c == 0), (c == j)
                    one = o8 if f8 else ob
                    for ss in range(2):
                        nc.tensor.matmul(rw[:], one[:], pT[:, ss],
                                         start=(first and ss == 0),
                                         stop=(last and ss == 1))
                    pts.append(pT)
                # H accumulation: per-cs-region groups kept contiguous (a
                # start=True clears has_written bank-wide, so interleaved
                # open groups in one bank do not accumulate correctly).
                for cs in range(CS):
                    for c in range(j + 1):
                        first, last = (c == 0), (c == j)
                        if f8:
                            nc.tensor.matmul(
                                po[:, cs], xn8_t[:, 2 * c:2 * c + 2, cs * P:(cs + 1) * P],
                                pts[c][:], start=first, stop=last, perf_mode=DR)
                        else:
                            for ss in range(2):
                                nc.tensor.matmul(
                                    po[:, cs], xnb_t[:, 2 * c + ss, cs * P:(cs + 1) * P],
                                    pts[c][:, ss], start=(first and ss == 0),
                                    stop=(last and ss == 1))
                # strip done: stash rowsum, cast H, pair-project.
                nc.vector.tensor_copy(rsum[:, j], rw[:])
                if j % 2 == 0:
                    hr = hrpool.tile([P, CS, 256], FP8 if f8 else BF16, tag="hr")
                nc.scalar.activation(
                    hr[:, :, (j % 2) * P:(j % 2) * P + P], po[:],
                    mybir.ActivationFunctionType.Copy,
                    scale=float(H_SCALE) if f8 else 1.0)
                if j % 2 == 1:
                    pr = j // 2
                    ost = opool.tile([P, CS, 256], BF16, tag="ost")
                    for ds in range(CS):
                        pf = psA.tile([P, 256], F32, tag="pq")
                        if f8:
                            for h in range(4):
                                nc.tensor.matmul(
                                    pf[:], wv8_t[:, 2 * h:2 * h + 2, ds * P:(ds + 1) * P],
                                    hr[:, 2 * h:2 * h + 2, :],
                                    start=(h == 0), stop=(h == 3), perf_mode=DR)
                        else:
                            for cs in range(CS):
                                nc.tensor.matmul(
                                    pf[:], wvb_t[:, cs, ds * P:(ds + 1) * P],
                                    hr[:, cs, :],
                                    start=(cs == 0), stop=(cs == CS - 1))
                        nc.vector.tensor_copy(ost[:, ds], pf[:])
                    nc.gpsimd.dma_start(
                        outT_d[:, :, 256 * pr:256 * (pr + 1)], ost[:])

            nc.sync.dma_start(rows_r[:], rsum[:])

    nc.compile()
    return nc


_NC = None


def _get_nc():
    global _NC
    if _NC is None:
        _NC = build()
    return _NC


def make_in_maps(x, Wq, Wk, Wv):
    x = np.asarray(x, dtype=np.float32)
    wq64 = np.asarray(Wq, np.float64)
    wk64 = np.asarray(Wk, np.float64)
    wm = (wq64.T @ wk64).astype(np.float32)
    wm_t = _tile128(wm.astype(NP_BF16))
    wvT = np.ascontiguousarray(np.asarray(Wv, np.float32).T)
    wvb_t = _tile128(wvT.astype(NP_BF16))
    wv8_t = _tile128((wvT * WV_SCALE).astype(NP_FP8))
    ones = np.ones((P, 1), np.float32)

    # additive masks [p(key sub), ss, t]: 0 = keep, MASK_NEG = drop.
    p_idx = np.arange(P)[:, None]
    t_idx = np.arange(P)[None, :]
    tri = np.where(p_idx <= t_idx, 0.0, MASK_NEG).astype(np.float32)
    zero = np.full((P, P), MASK_NEG, np.float32)
    full = np.zeros((P, P), np.float32)
    mask_A = np.stack([tri, zero], axis=1)
    mask_B = np.stack([full, tri], axis=1)

    in_maps = []
    for core in range(8):
        b, role = divmod(core, 2)
        xb = x[b]
        xTb = np.ascontiguousarray(xb.T).astype(NP_BF16)
        cols = np.concatenate(
            [np.arange(256 * j + P * role, 256 * j + P * role + P)
             for j in range(NS)])
        in_maps.append({
            "xt": _tile128(xTb),
            "xq": _tile128(np.ascontiguousarray(xTb[:, cols])),
            "wm": wm_t,
            "xnb": _tile128(xb[0:512].astype(NP_BF16)),
            "xn8": _tile128(xb.astype(NP_FP8)),
            "wvb": wvb_t,
            "wv8": wv8_t,
            "msk": (mask_A if role == 0 else mask_B).astype(NP_BF16),
            "onesb": ones.astype(NP_BF16),
            "ones8": ones.astype(NP_FP8),
            "biasv": np.full((P, 1), EXP_BIAS, np.float32),
        })
    return in_maps


def assemble(results):
    out = np.empty((B, T, C), np.float32)
    for core in range(8):
        b, role = divmod(core, 2)
        oTt = np.asarray(results[core]["outTb"]).astype(np.float32)  # [128, 8, 1024]
        oT = oTt.transpose(1, 0, 2).reshape(C, NS * P)
        rs = np.asarray(results[core]["rows"]).reshape(NS * P)
        for j in range(NS):
            sl = slice(j * P, (j + 1) * P)
            o = oT[:, sl].T / rs[sl][:, None]
            if j >= FP8_FROM:
                o = o * OUT_RESCALE
            r0 = 256 * j + P * role
            out[b, r0:r0 + P] = o
    return out


def kernel(x, Wq, Wk, Wv):
    nc = _get_nc()
    in_maps = make_in_maps(x, Wq, Wk, Wv)
    res = bass_utils.run_bass_kernel_spmd(nc, in_maps, core_ids=list(range(8)))
    return assemble(res.results)


def _install_trace_shim():
    """Provide antenv.axon_hooks (absent in this image) so trace=True works."""
    import sys
    import types
    if "antenv.axon_hooks" in sys.modules:
        return
    hook_box = [None]
    mod = types.ModuleType("antenv.axon_hooks")
    mod.set_axon_ntff_profile_hook = lambda h: hook_box.__setitem__(0, h)
    mod.get_axon_ntff_profile_hook = lambda: hook_box[0]
    import antenv
    sys.modules["antenv.axon_hooks"] = mod
    antenv.axon_hooks = mod
    try:
        from trn_agent_boot.trn_boot import _ntff_profile_via_ctypes
        mod.set_axon_ntff_profile_hook(
            _ntff_profile_via_ctypes("/opt/axon/libaxon_pjrt.so"))
    except Exception:
        pass


def run_traced(x, Wq, Wk, Wv):
    """Like kernel() but with NTFF tracing; returns (out, BassKernelResults)."""
    _install_trace_shim()
    nc = _get_nc()
    in_maps = make_in_maps(x, Wq, Wk, Wv)
    res = bass_utils.run_bass_kernel_spmd(
        nc, in_maps, core_ids=list(range(8)), trace=True,
        trace_cores=list(range(8)))
    return assemble(res.results), res


# revision 11
# speedup vs baseline: 1.1332x; 1.1089x over previous

"""Causal attention (no head split) on 8 trn2 NeuronCores — v2.

Math (per batch b), K/V never materialized:
    S   = x M x^T with M = Wq^T Wk precomputed on host
    out = (wei x) Wv^T; device computes H = x^T wei^T then O^T = Wv H.

v2 structure (vs v1):
  * strip-major schedule: 8 query strips of 128 rows per core; strip j
    attends key chunks 0..j (256 keys each). Exact causal tiling for both
    roles (role A takes even 128-row blocks, role B odd) — zero fully
    masked visits.
  * logit path bf16; strips >= FP8_FROM run P/xn/H-cast/Wv in fp8 e4m3
    with DoubleRow matmuls; earlier strips (small softmax rows) stay bf16.
  * H and rowsum accumulate in PSUM across a strip's chunks.
  * causal mask applied additively to S *before* exp (no fp8 inf*0).
  * all inputs pre-tiled on host to [128, X] contiguous layouts; one DMA
    per tensor half, staggered so the G-phase gate (wm, xq) lands first.
"""
import numpy as np
import ml_dtypes

import concourse.bass as bass
from concourse import bacc
import concourse.mybir as mybir
from concourse.tile import TileContext
from concourse import bass_utils

B, T, C = 4, 2048, 1024
P = 128
CS = C // P           # 8 contraction subtiles
NS = 8                # strips per core (128 queries each)
SCALE = 1.0 / np.sqrt(C)   # 1/32
EXP_BIAS = -2.5
FP8_FROM = 2          # strips >= this use fp8 path
H_SCALE = 0.25        # H -> fp8 cast scale
WV_SCALE = 64.0       # Wv fp8 scale
OUT_RESCALE = 1.0 / (H_SCALE * WV_SCALE)
MASK_NEG = -65536.0   # additive mask for non-causal logits

BF16 = mybir.dt.bfloat16
FP8 = mybir.dt.float8e4
F32 = mybir.dt.float32
DR = mybir.MatmulPerfMode.DoubleRow

NP_BF16 = ml_dtypes.bfloat16
NP_FP8 = ml_dtypes.float8_e4m3


def _tile128(a):
    """[(g*128), X] row-major -> [128, g, X] (partition-major pre-tiling)."""
    g = a.shape[0] // P
    return np.ascontiguousarray(a.reshape(g, P, -1).transpose(1, 0, 2))


def build():
    nc = bacc.Bacc(trn_type="TRN2", name="causal_attn_v2")
    xt_d = [nc.dram_tensor(f"xt{k}", [P, CS, 512], BF16, kind="ExternalInput")
            for k in range(4)]
    xq0_d = nc.dram_tensor("xq0", [P, CS, 256], BF16, kind="ExternalInput")
    xq13_d = nc.dram_tensor("xq13", [P, CS, 768], BF16, kind="ExternalInput")
    wm0_d = nc.dram_tensor("wm0", [P, CS, P], BF16, kind="ExternalInput")
    wm13_d = nc.dram_tensor("wm13", [P, CS, 384], BF16, kind="ExternalInput")
    wm47_d = nc.dram_tensor("wm47", [P, CS, 512], BF16, kind="ExternalInput")
    xnb_d = nc.dram_tensor("xnb", [P, 4, C], BF16, kind="ExternalInput")
    xn8_d = [nc.dram_tensor(f"xn8{k}", [P, 8, C], FP8, kind="ExternalInput")
             for k in range(2)]
    wvb_d = nc.dram_tensor("wvb", [P, CS, C], BF16, kind="ExternalInput")
    wv8_d = nc.dram_tensor("wv8", [P, CS, C], FP8, kind="ExternalInput")
    msk_d = nc.dram_tensor("msk", [P, 2, P], BF16, kind="ExternalInput")
    onesb_d = nc.dram_tensor("onesb", [P, 1], BF16, kind="ExternalInput")
    ones8_d = nc.dram_tensor("ones8", [P, 1], FP8, kind="ExternalInput")
    biasv_d = nc.dram_tensor("biasv", [P, 1], F32, kind="ExternalInput")
    outT_d = nc.dram_tensor("outTb", [P, 4, CS, 256], BF16, kind="ExternalOutput")
    rows_d = nc.dram_tensor("rows", [1, NS * P], F32, kind="ExternalOutput")
    rows_r = rows_d.rearrange("p (j t) -> p j t", j=NS)

    with TileContext(nc) as tc:
        with tc.tile_pool(name="keep", bufs=1) as keep, \
             tc.tile_pool(name="hrpool", bufs=2) as hrpool, \
             tc.tile_pool(name="opool", bufs=2) as opool, \
             tc.tile_pool(name="ppool", bufs=10) as ppool, \
             tc.tile_pool(name="psA", bufs=2, space="PSUM") as psA, \
             tc.tile_pool(name="psS", bufs=3, space="PSUM") as psS, \
             tc.tile_pool(name="psH", bufs=1, space="PSUM") as psH, \
             tc.tile_pool(name="psR", bufs=1, space="PSUM") as psR:

            # ---------------- resident tiles ----------------
            xt = [keep.tile([P, CS, 512], BF16, name=f"xt{k}", tag=f"xt{k}") for k in range(4)]
            xq0 = keep.tile([P, CS, 256], BF16, tag="xq0")
            xq13 = keep.tile([P, CS, 768], BF16, tag="xq13")
            wm0 = keep.tile([P, CS, P], BF16, tag="wm0")
            wm13 = keep.tile([P, CS, 384], BF16, tag="wm13")
            wm47 = keep.tile([P, CS, 512], BF16, tag="wm47")
            gT = keep.tile([P, CS, NS * P], BF16, tag="gT")     # 16KB
            xnb_t = keep.tile([P, 4, C], BF16, tag="xnb")       # 8KB
            xn8_t = [keep.tile([P, 8, C], FP8, name=f"xn8{k}", tag=f"xn8{k}") for k in range(2)]
            wvb_t = keep.tile([P, CS, C], BF16, tag="wvb")      # 16KB
            wv8_t = keep.tile([P, CS, C], FP8, tag="wv8")       # 8KB
            mt = keep.tile([P, 2, P], BF16, tag="mt")
            ob = keep.tile([P, 1], BF16, tag="ob")
            o8 = keep.tile([P, 1], FP8, tag="o8")
            bv = keep.tile([P, 1], F32, tag="bv")
            rsum = keep.tile([1, NS, P], F32, tag="rsum")

            # -------- DMAs: whole-tensor (contiguous), 3 queues ---------
            # sync starts with the smallest G-gate piece; scalar brings the
            # query columns in parallel; gpsimd takes constants + values.
            nc.sync.dma_start(wm0[:], wm0_d[:])
            nc.sync.dma_start(wm13[:], wm13_d[:])
            nc.sync.dma_start(wm47[:], wm47_d[:])
            for k in range(4):
                nc.sync.dma_start(xt[k][:], xt_d[k][:])
            nc.scalar.dma_start(xq0[:], xq0_d[:])
            nc.scalar.dma_start(xq13[:], xq13_d[:])
            nc.scalar.dma_start(wvb_t[:], wvb_d[:])
            nc.scalar.dma_start(wv8_t[:], wv8_d[:])
            nc.gpsimd.dma_start(mt[:], msk_d[:])
            nc.gpsimd.dma_start(ob[:], onesb_d[:])
            nc.gpsimd.dma_start(o8[:], ones8_d[:])
            nc.gpsimd.dma_start(bv[:], biasv_d[:])
            nc.gpsimd.dma_start(xnb_t[:], xnb_d[:])
            nc.gpsimd.dma_start(xn8_t[0][:], xn8_d[0][:])
            nc.gpsimd.dma_start(xn8_t[1][:], xn8_d[1][:])

            # ---------------- Phase G: gT = M^T xq^T (bf16) -------------
            def wm_sl(ds, cs):
                if ds == 0:
                    return wm0[:, cs, :]
                if ds < 4:
                    return wm13[:, cs, (ds - 1) * P:ds * P]
                return wm47[:, cs, (ds - 4) * P:(ds - 3) * P]

            for q in range(4):
                qs = slice(256 * q, 256 * (q + 1))
                for ds in range(CS):
                    pq = psA.tile([P, 256], F32, tag="pq")
                    for cs in range(CS):
                        xq_sl = (xq0[:, cs, :] if q == 0
                                 else xq13[:, cs, (q - 1) * 256:q * 256])
                        nc.tensor.matmul(
                            pq[:], wm_sl(ds, cs), xq_sl,
                            start=(cs == 0), stop=(cs == CS - 1))
                    nc.vector.tensor_copy(gT[:, ds, qs], pq[:])

            # ---------------- strip-major attention ---------------------
            hr = None
            for j in range(NS):
                f8 = j >= FP8_FROM
                tsl = slice(j * P, (j + 1) * P)
                po = psH.tile([P, CS, P], F32, tag="po")
                rw = psR.tile([1, P], F32, tag="rw")
                pts = []
                for c in range(j + 1):
                    st = psS.tile([P, 2, P], F32, tag="st")
                    for ss in range(2):
                        for cs in range(CS):
                            off = 256 * (c % 2) + P * ss
                            nc.tensor.matmul(
                                st[:, ss],
                                xt[c // 2][:, cs, off:off + P],
                                gT[:, cs, tsl],
                                start=(cs == 0), stop=(cs == CS - 1))
                    if c == j:  # additive causal mask on the diagonal chunk
                        nc.vector.tensor_add(st[:], st[:], mt[:])
                    pT = ppool.tile([P, 2, P], FP8 if f8 else BF16, tag="pT")
                    nc.scalar.activation(
                        pT[:], st[:], mybir.ActivationFunctionType.Exp,
                        scale=float(SCALE), bias=bv[:])
                    first, last = (c == 0), (c == j)
                    one = o8 if f8 else ob
                    for ss in range(2):
                        nc.tensor.matmul(rw[:], one[:], pT[:, ss],
                                         start=(first and ss == 0),
                                         stop=(last and ss == 1))
                    pts.append(pT)
                # H accumulation: per-cs-region groups kept contiguous (a
                # start=True clears has_written bank-wide, so interleaved
                # open groups in one bank do not accumulate correctly).
                for cs in range(CS):
                    for c in range(j + 1):
                        first, last = (c == 0), (c == j)
                        if f8:
                            hh = (2 * c) % 8
                            nc.tensor.matmul(
                                po[:, cs], xn8_t[c // 4][:, hh:hh + 2, cs * P:(cs + 1) * P],
                                pts[c][:], start=first, stop=last, perf_mode=DR)
                        else:
                            for ss in range(2):
                                nc.tensor.matmul(
                                    po[:, cs], xnb_t[:, 2 * c + ss, cs * P:(cs + 1) * P],
                                    pts[c][:, ss], start=(first and ss == 0),
                                    stop=(last and ss == 1))
                # strip done: stash rowsum, cast H, pair-project.
                nc.vector.tensor_copy(rsum[:, j], rw[:])
                if j == NS - 1:
                    nc.sync.dma_start(rows_r[:], rsum[:])
                if j % 2 == 0:
                    hr = hrpool.tile([P, CS, 256], FP8 if f8 else BF16, tag="hr")
                nc.scalar.activation(
                    hr[:, :, (j % 2) * P:(j % 2) * P + P], po[:],
                    mybir.ActivationFunctionType.Copy,
                    scale=float(H_SCALE) if f8 else 1.0)
                if j % 2 == 1:
                    pr = j // 2
                    ost = opool.tile([P, CS, 256], BF16, tag="ost")
                    for ds in range(CS):
                        pf = psA.tile([P, 256], F32, tag="pq")
                        if f8:
                            for h in range(4):
                                nc.tensor.matmul(
                                    pf[:], wv8_t[:, 2 * h:2 * h + 2, ds * P:(ds + 1) * P],
                                    hr[:, 2 * h:2 * h + 2, :],
                                    start=(h == 0), stop=(h == 3), perf_mode=DR)
                        else:
                            for cs in range(CS):
                                nc.tensor.matmul(
                                    pf[:], wvb_t[:, cs, ds * P:(ds + 1) * P],
                                    hr[:, cs, :],
                                    start=(cs == 0), stop=(cs == CS - 1))
                        nc.vector.tensor_copy(ost[:, ds], pf[:])
                    nc.gpsimd.dma_start(outT_d[:, pr], ost[:])

    nc.compile()
    return nc


_NC = None


def _get_nc():
    global _NC
    if _NC is None:
        _NC = build()
    return _NC


def make_in_maps(x, Wq, Wk, Wv):
    x = np.asarray(x, dtype=np.float32)
    wq64 = np.asarray(Wq, np.float64)
    wk64 = np.asarray(Wk, np.float64)
    wm = (wq64.T @ wk64).astype(np.float32)
    wm_t = _tile128(wm.astype(NP_BF16))
    wvT = np.ascontiguousarray(np.asarray(Wv, np.float32).T)
    wvb_t = _tile128(wvT.astype(NP_BF16))
    wv8_t = _tile128((wvT * WV_SCALE).astype(NP_FP8))
    ones = np.ones((P, 1), np.float32)

    # additive masks [p(key sub), ss, t]: 0 = keep, MASK_NEG = drop.
    p_idx = np.arange(P)[:, None]
    t_idx = np.arange(P)[None, :]
    tri = np.where(p_idx <= t_idx, 0.0, MASK_NEG).astype(np.float32)
    zero = np.full((P, P), MASK_NEG, np.float32)
    full = np.zeros((P, P), np.float32)
    mask_A = np.stack([tri, zero], axis=1)
    mask_B = np.stack([full, tri], axis=1)

    in_maps = []
    for core in range(8):
        b, role = divmod(core, 2)
        xb = x[b]
        xTb = np.ascontiguousarray(xb.T).astype(NP_BF16)
        cols = np.concatenate(
            [np.arange(256 * j + P * role, 256 * j + P * role + P)
             for j in range(NS)])
        xt_t = _tile128(xTb)                     # [128, 8, 2048]
        xq_t = _tile128(np.ascontiguousarray(xTb[:, cols]))
        xn8_t = _tile128(xb.astype(NP_FP8))      # [128, 16, 1024]
        in_maps.append({
            **{f"xt{k}": np.ascontiguousarray(xt_t[:, :, 512 * k:512 * (k + 1)])
               for k in range(4)},
            "xq0": np.ascontiguousarray(xq_t[:, :, 0:256]),
            "xq13": np.ascontiguousarray(xq_t[:, :, 256:1024]),
            "wm0": np.ascontiguousarray(wm_t[:, :, 0:128]),
            "wm13": np.ascontiguousarray(wm_t[:, :, 128:512]),
            "wm47": np.ascontiguousarray(wm_t[:, :, 512:1024]),
            "xnb": _tile128(xb[0:512].astype(NP_BF16)),
            "xn80": np.ascontiguousarray(xn8_t[:, 0:8]),
            "xn81": np.ascontiguousarray(xn8_t[:, 8:16]),
            "wvb": wvb_t,
            "wv8": wv8_t,
            "msk": (mask_A if role == 0 else mask_B).astype(NP_BF16),
            "onesb": ones.astype(NP_BF16),
            "ones8": ones.astype(NP_FP8),
            "biasv": np.full((P, 1), EXP_BIAS, np.float32),
        })
    return in_maps


def assemble(results):
    out = np.empty((B, T, C), np.float32)
    for core in range(8):
        b, role = divmod(core, 2)
        oTt = np.asarray(results[core]["outTb"]).astype(np.float32)  # [128,4,8,256]
        oT = oTt.transpose(2, 0, 1, 3).reshape(C, NS * P)
        rs = np.asarray(results[core]["rows"]).reshape(NS * P)
        for j in range(NS):
            sl = slice(j * P, (j + 1) * P)
            o = oT[:, sl].T / rs[sl][:, None]
            if j >= FP8_FROM:
                o = o * OUT_RESCALE
            r0 = 256 * j + P * role
            out[b, r0:r0 + P] = o
    return out


def kernel(x, Wq, Wk, Wv):
    nc = _get_nc()
    in_maps = make_in_maps(x, Wq, Wk, Wv)
    res = bass_utils.run_bass_kernel_spmd(nc, in_maps, core_ids=list(range(8)))
    return assemble(res.results)


def _install_trace_shim():
    """Provide antenv.axon_hooks (absent in this image) so trace=True works."""
    import sys
    import types
    if "antenv.axon_hooks" in sys.modules:
        return
    hook_box = [None]
    mod = types.ModuleType("antenv.axon_hooks")
    mod.set_axon_ntff_profile_hook = lambda h: hook_box.__setitem__(0, h)
    mod.get_axon_ntff_profile_hook = lambda: hook_box[0]
    import antenv
    sys.modules["antenv.axon_hooks"] = mod
    antenv.axon_hooks = mod
    try:
        from trn_agent_boot.trn_boot import _ntff_profile_via_ctypes
        mod.set_axon_ntff_profile_hook(
            _ntff_profile_via_ctypes("/opt/axon/libaxon_pjrt.so"))
    except Exception:
        pass


def run_traced(x, Wq, Wk, Wv):
    """Like kernel() but with NTFF tracing; returns (out, BassKernelResults)."""
    _install_trace_shim()
    nc = _get_nc()
    in_maps = make_in_maps(x, Wq, Wk, Wv)
    res = bass_utils.run_bass_kernel_spmd(
        nc, in_maps, core_ids=list(range(8)), trace=True,
        trace_cores=list(range(8)))
    return assemble(res.results), res


# revision 14
# speedup vs baseline: 1.2255x; 1.0815x over previous

"""Causal attention (no head split) on 8 trn2 NeuronCores — v2.

Math (per batch b), K/V never materialized:
    S   = x M x^T with M = Wq^T Wk precomputed on host
    out = (wei x) Wv^T; device computes H = x^T wei^T then O^T = Wv H.

v2 structure (vs v1):
  * strip-major schedule: 8 query strips of 128 rows per core; strip j
    attends key chunks 0..j (256 keys each). Exact causal tiling for both
    roles (role A takes even 128-row blocks, role B odd) — zero fully
    masked visits.
  * logit path bf16; strips >= FP8_FROM run P/xn/H-cast/Wv in fp8 e4m3
    with DoubleRow matmuls; earlier strips (small softmax rows) stay bf16.
  * H and rowsum accumulate in PSUM across a strip's chunks.
  * causal mask applied additively to S *before* exp (no fp8 inf*0).
  * all inputs pre-tiled on host to [128, X] contiguous layouts; one DMA
    per tensor half, staggered so the G-phase gate (wm, xq) lands first.
"""
import numpy as np
import ml_dtypes

import concourse.bass as bass
from concourse import bacc
import concourse.mybir as mybir
from concourse.tile import TileContext
from concourse import bass_utils

B, T, C = 4, 2048, 1024
P = 128
CS = C // P           # 8 contraction subtiles
NS = 8                # strips per core (128 queries each)
SCALE = 1.0 / np.sqrt(C)   # 1/32
EXP_BIAS = -2.5
FP8_FROM = 2          # strips >= this use fp8 path
H_SCALE = 0.25        # H -> fp8 cast scale
WV_SCALE = 64.0       # Wv fp8 scale
OUT_RESCALE = 1.0 / (H_SCALE * WV_SCALE)
MASK_NEG = -65536.0   # additive mask for non-causal logits

BF16 = mybir.dt.bfloat16
FP8 = mybir.dt.float8e4
F32 = mybir.dt.float32
DR = mybir.MatmulPerfMode.DoubleRow

NP_BF16 = ml_dtypes.bfloat16
NP_FP8 = ml_dtypes.float8_e4m3


def _tile128(a):
    """[(g*128), X] row-major -> [128, g, X] (partition-major pre-tiling)."""
    g = a.shape[0] // P
    return np.ascontiguousarray(a.reshape(g, P, -1).transpose(1, 0, 2))


def build():
    nc = bacc.Bacc(trn_type="TRN2", name="causal_attn_v2")
    xt_d = [nc.dram_tensor(f"xt{k}", [P, CS, 512], BF16, kind="ExternalInput")
            for k in range(4)]
    xq0_d = nc.dram_tensor("xq0", [P, CS, 256], BF16, kind="ExternalInput")
    xq13_d = nc.dram_tensor("xq13", [P, CS, 768], BF16, kind="ExternalInput")
    wm0_d = nc.dram_tensor("wm0", [P, CS, P], BF16, kind="ExternalInput")
    wm13_d = nc.dram_tensor("wm13", [P, CS, 384], BF16, kind="ExternalInput")
    wm47_d = nc.dram_tensor("wm47", [P, CS, 512], BF16, kind="ExternalInput")
    xnb_d = nc.dram_tensor("xnb", [P, 4, C], BF16, kind="ExternalInput")
    xn8_d = [nc.dram_tensor(f"xn8{k}", [P, 8, C], FP8, kind="ExternalInput")
             for k in range(2)]
    wvb_d = nc.dram_tensor("wvb", [P, CS, C], BF16, kind="ExternalInput")
    wv8_d = nc.dram_tensor("wv8", [P, CS, C], FP8, kind="ExternalInput")
    msk_d = nc.dram_tensor("msk", [P, 2, P], BF16, kind="ExternalInput")
    onesb_d = nc.dram_tensor("onesb", [P, 1], BF16, kind="ExternalInput")
    ones8_d = nc.dram_tensor("ones8", [P, 1], FP8, kind="ExternalInput")
    biasv_d = nc.dram_tensor("biasv", [P, 1], F32, kind="ExternalInput")
    outT_d = nc.dram_tensor("outTb", [P, 4, CS, 256], BF16, kind="ExternalOutput")
    rows_d = nc.dram_tensor("rows", [1, NS * P], F32, kind="ExternalOutput")
    rows_r = rows_d.rearrange("p (j t) -> p j t", j=NS)

    with TileContext(nc) as tc:
        with tc.tile_pool(name="keep", bufs=1) as keep, \
             tc.tile_pool(name="hrpool", bufs=2) as hrpool, \
             tc.tile_pool(name="opool", bufs=2) as opool, \
             tc.tile_pool(name="ppool", bufs=10) as ppool, \
             tc.tile_pool(name="psA", bufs=2, space="PSUM") as psA, \
             tc.tile_pool(name="psS", bufs=2, space="PSUM") as psS, \
             tc.tile_pool(name="psH", bufs=1, space="PSUM") as psH, \
             tc.tile_pool(name="psR", bufs=1, space="PSUM") as psR, \
             tc.tile_pool(name="psR2", bufs=1, space="PSUM") as psR2:

            # ---------------- resident tiles ----------------
            xt = [keep.tile([P, CS, 512], BF16, name=f"xt{k}", tag=f"xt{k}") for k in range(4)]
            xq0 = keep.tile([P, CS, 256], BF16, tag="xq0")
            xq13 = keep.tile([P, CS, 768], BF16, tag="xq13")
            wm0 = keep.tile([P, CS, P], BF16, tag="wm0")
            wm13 = keep.tile([P, CS, 384], BF16, tag="wm13")
            wm47 = keep.tile([P, CS, 512], BF16, tag="wm47")
            gT = keep.tile([P, CS, NS * P], BF16, tag="gT")     # 16KB
            xnb_t = keep.tile([P, 4, C], BF16, tag="xnb")       # 8KB
            xn8_t = [keep.tile([P, 8, C], FP8, name=f"xn8{k}", tag=f"xn8{k}") for k in range(2)]
            wvb_t = keep.tile([P, CS, C], BF16, tag="wvb")      # 16KB
            wv8_t = keep.tile([P, CS, C], FP8, tag="wv8")       # 8KB
            mt = keep.tile([P, 2, P], BF16, tag="mt")
            ob = keep.tile([P, 1], BF16, tag="ob")
            o8 = keep.tile([P, 1], FP8, tag="o8")
            bv = keep.tile([P, 1], F32, tag="bv")
            rsum = keep.tile([1, NS, P], F32, tag="rsum")

            warm = keep.tile([P, 256], BF16, tag="warm")
            # -------- DMAs: whole-tensor (contiguous), 3 queues ---------
            # G-gate pieces split across all three queues for parallel
            # transfer; bulk follows in need order.
            nc.sync.dma_start(wm0[:], wm0_d[:])
            nc.sync.dma_start(wm13[:], wm13_d[:])
            for k in range(4):
                nc.sync.dma_start(xt[k][:], xt_d[k][:])
            nc.scalar.dma_start(xq0[:], xq0_d[:])
            nc.scalar.dma_start(xq13[:], xq13_d[:])
            nc.scalar.dma_start(wvb_t[:], wvb_d[:])
            nc.scalar.dma_start(wv8_t[:], wv8_d[:])
            nc.gpsimd.dma_start(wm47[:], wm47_d[:])
            nc.gpsimd.dma_start(mt[:], msk_d[:])
            nc.gpsimd.dma_start(ob[:], onesb_d[:])
            nc.gpsimd.dma_start(o8[:], ones8_d[:])
            nc.gpsimd.dma_start(bv[:], biasv_d[:])
            nc.gpsimd.dma_start(xnb_t[:], xnb_d[:])
            nc.gpsimd.dma_start(xn8_t[0][:], xn8_d[0][:])
            nc.gpsimd.dma_start(xn8_t[1][:], xn8_d[1][:])

            # -------- PE prewarm: dummy matmuls while DMAs land ---------
            # The HAM clock-gate needs ~3.4us of sustained PE activity to
            # un-throttle 1.2 -> 2.4 GHz; run throwaway matmuls on a
            # memset tile so the G phase starts at full clock.
            nc.gpsimd.memset(warm[:], 0.0)
            pw = None
            for w in range(32):
                pw = psA.tile([P, 256], F32, tag="pq")
                nc.tensor.matmul(pw[:], warm[:, 0:P], warm[:],
                                 start=True, stop=True)
            nc.vector.tensor_copy(warm[:], pw[:])

            # ---------------- Phase G: gT = M^T xq^T (bf16) -------------
            def wm_sl(ds, cs):
                if ds == 0:
                    return wm0[:, cs, :]
                if ds < 4:
                    return wm13[:, cs, (ds - 1) * P:ds * P]
                return wm47[:, cs, (ds - 4) * P:(ds - 3) * P]

            for q in range(4):
                qs = slice(256 * q, 256 * (q + 1))
                for ds in range(CS):
                    pq = psA.tile([P, 256], F32, tag="pq")
                    for cs in range(CS):
                        xq_sl = (xq0[:, cs, :] if q == 0
                                 else xq13[:, cs, (q - 1) * 256:q * 256])
                        nc.tensor.matmul(
                            pq[:], wm_sl(ds, cs), xq_sl,
                            start=(cs == 0), stop=(cs == CS - 1))
                    nc.vector.tensor_copy(gT[:, ds, qs], pq[:])

            # ---------------- strip-pair attention ----------------------
            # Strips 2k and 2k+1 share key chunks 0..2k: process them as a
            # 256-column pair (S/exp at 256 cols), then one 128-col visit
            # for strip 2k+1's diagonal chunk. Zero masked-visit waste.
            for k in range(4):
                j, j2 = 2 * k, 2 * k + 1
                f8 = j >= FP8_FROM
                one = o8 if f8 else ob
                pdt = FP8 if f8 else BF16
                cols = slice(j * P, j * P + 256)
                rw = psR.tile([1, 256], F32, tag="rw")
                pts = []
                for c in range(j + 1):
                    st = psS.tile([P, 2, 256], F32, tag="st")
                    for ss in range(2):
                        for cs in range(CS):
                            off = 256 * (c % 2) + P * ss
                            nc.tensor.matmul(
                                st[:, ss], xt[c // 2][:, cs, off:off + P],
                                gT[:, cs, cols],
                                start=(cs == 0), stop=(cs == CS - 1))
                    if c == j:  # diagonal of strip 2k: mask its 128 cols
                        nc.vector.tensor_add(st[:, :, 0:P], st[:, :, 0:P], mt[:])
                    pT = ppool.tile([P, 2, 256], pdt, tag="pT")
                    nc.scalar.activation(
                        pT[:], st[:], mybir.ActivationFunctionType.Exp,
                        scale=float(SCALE), bias=bv[:])
                    for ss in range(2):
                        nc.tensor.matmul(rw[:], one[:], pT[:, ss],
                                         start=(c == 0 and ss == 0),
                                         stop=(c == j and ss == 1))
                    pts.append(pT)
                # H burst for strip 2k (even): chunks 0..j complete.
                po = psH.tile([P, CS, P], F32, tag="po")
                for cs in range(CS):
                    for c in range(j + 1):
                        first, last = (c == 0), (c == j)
                        if f8:
                            hh = (2 * c) % 8
                            nc.tensor.matmul(
                                po[:, cs],
                                xn8_t[c // 4][:, hh:hh + 2, cs * P:(cs + 1) * P],
                                pts[c][:, 0:2, 0:P],
                                start=first, stop=last, perf_mode=DR)
                        else:
                            for ss in range(2):
                                nc.tensor.matmul(
                                    po[:, cs],
                                    xnb_t[:, 2 * c + ss, cs * P:(cs + 1) * P],
                                    pts[c][:, ss, 0:P],
                                    start=(first and ss == 0),
                                    stop=(last and ss == 1))
                nc.vector.tensor_copy(rsum[:, j], rw[:, 0:P])
                nc.vector.tensor_copy(rsum[:, j2], rw[:, P:256])
                hr = hrpool.tile([P, CS, 256], pdt, tag="hr")
                nc.scalar.activation(
                    hr[:, :, 0:P], po[:],
                    mybir.ActivationFunctionType.Copy,
                    scale=float(H_SCALE) if f8 else 1.0)

                # diagonal visit for strip 2k+1 (chunk j+1, 128 cols)
                c = j + 1
                st2 = psS.tile([P, 2, P], F32, tag="st")
                for ss in range(2):
                    for cs in range(CS):
                        off = 256 * (c % 2) + P * ss
                        nc.tensor.matmul(
                            st2[:, ss], xt[c // 2][:, cs, off:off + P],
                            gT[:, cs, j2 * P:(j2 + 1) * P],
                            start=(cs == 0), stop=(cs == CS - 1))
                nc.vector.tensor_add(st2[:], st2[:], mt[:])
                pT2 = ppool.tile([P, 2, P], pdt, tag="pT")
                nc.scalar.activation(
                    pT2[:], st2[:], mybir.ActivationFunctionType.Exp,
                    scale=float(SCALE), bias=bv[:])
                rw2 = psR2.tile([1, P], F32, tag="rw2")
                for ss in range(2):
                    nc.tensor.matmul(rw2[:], one[:], pT2[:, ss],
                                     start=(ss == 0), stop=(ss == 1))
                pts.append(pT2)
                # H burst for strip 2k+1: chunks 0..j from the pair pT's
                # right half + the diagonal pT2.
                po2 = psH.tile([P, CS, P], F32, tag="po")
                for cs in range(CS):
                    for c in range(j + 2):
                        first, last = (c == 0), (c == j + 1)
                        psl = pts[c][:, 0:2, P:256] if c <= j else pts[c][:, 0:2, 0:P]
                        if f8:
                            hh = (2 * c) % 8
                            nc.tensor.matmul(
                                po2[:, cs],
                                xn8_t[c // 4][:, hh:hh + 2, cs * P:(cs + 1) * P],
                                psl, start=first, stop=last, perf_mode=DR)
                        else:
                            for ss in range(2):
                                nc.tensor.matmul(
                                    po2[:, cs],
                                    xnb_t[:, 2 * c + ss, cs * P:(cs + 1) * P],
                                    psl[:, ss], start=(first and ss == 0),
                                    stop=(last and ss == 1))
                nc.vector.tensor_add(rsum[:, j2], rsum[:, j2], rw2[:])
                if k == 3:
                    nc.sync.dma_start(rows_r[:], rsum[:])
                nc.scalar.activation(
                    hr[:, :, P:256], po2[:],
                    mybir.ActivationFunctionType.Copy,
                    scale=float(H_SCALE) if f8 else 1.0)

                # pair projection
                ost = opool.tile([P, CS, 256], BF16, tag="ost")
                for ds in range(CS):
                    pf = psA.tile([P, 256], F32, tag="pq")
                    if f8:
                        for h in range(4):
                            nc.tensor.matmul(
                                pf[:], wv8_t[:, 2 * h:2 * h + 2, ds * P:(ds + 1) * P],
                                hr[:, 2 * h:2 * h + 2, :],
                                start=(h == 0), stop=(h == 3), perf_mode=DR)
                    else:
                        for cs in range(CS):
                            nc.tensor.matmul(
                                pf[:], wvb_t[:, cs, ds * P:(ds + 1) * P],
                                hr[:, cs, :],
                                start=(cs == 0), stop=(cs == CS - 1))
                    nc.vector.tensor_copy(ost[:, ds], pf[:])
                nc.gpsimd.dma_start(outT_d[:, k], ost[:])

    nc.compile()
    return nc


_NC = None


def _get_nc():
    global _NC
    if _NC is None:
        _NC = build()
    return _NC


def make_in_maps(x, Wq, Wk, Wv):
    x = np.asarray(x, dtype=np.float32)
    wq64 = np.asarray(Wq, np.float64)
    wk64 = np.asarray(Wk, np.float64)
    wm = (wq64.T @ wk64).astype(np.float32)
    wm_t = _tile128(wm.astype(NP_BF16))
    wvT = np.ascontiguousarray(np.asarray(Wv, np.float32).T)
    wvb_t = _tile128(wvT.astype(NP_BF16))
    wv8_t = _tile128((wvT * WV_SCALE).astype(NP_FP8))
    ones = np.ones((P, 1), np.float32)

    # additive masks [p(key sub), ss, t]: 0 = keep, MASK_NEG = drop.
    p_idx = np.arange(P)[:, None]
    t_idx = np.arange(P)[None, :]
    tri = np.where(p_idx <= t_idx, 0.0, MASK_NEG).astype(np.float32)
    zero = np.full((P, P), MASK_NEG, np.float32)
    full = np.zeros((P, P), np.float32)
    mask_A = np.stack([tri, zero], axis=1)
    mask_B = np.stack([full, tri], axis=1)

    in_maps = []
    for core in range(8):
        b, role = divmod(core, 2)
        xb = x[b]
        xTb = np.ascontiguousarray(xb.T).astype(NP_BF16)
        cols = np.concatenate(
            [np.arange(256 * j + P * role, 256 * j + P * role + P)
             for j in range(NS)])
        xt_t = _tile128(xTb)                     # [128, 8, 2048]
        xq_t = _tile128(np.ascontiguousarray(xTb[:, cols]))
        xn8_t = _tile128(xb.astype(NP_FP8))      # [128, 16, 1024]
        in_maps.append({
            **{f"xt{k}": np.ascontiguousarray(xt_t[:, :, 512 * k:512 * (k + 1)])
               for k in range(4)},
            "xq0": np.ascontiguousarray(xq_t[:, :, 0:256]),
            "xq13": np.ascontiguousarray(xq_t[:, :, 256:1024]),
            "wm0": np.ascontiguousarray(wm_t[:, :, 0:128]),
            "wm13": np.ascontiguousarray(wm_t[:, :, 128:512]),
            "wm47": np.ascontiguousarray(wm_t[:, :, 512:1024]),
            "xnb": _tile128(xb[0:512].astype(NP_BF16)),
            "xn80": np.ascontiguousarray(xn8_t[:, 0:8]),
            "xn81": np.ascontiguousarray(xn8_t[:, 8:16]),
            "wvb": wvb_t,
            "wv8": wv8_t,
            "msk": (mask_A if role == 0 else mask_B).astype(NP_BF16),
            "onesb": ones.astype(NP_BF16),
            "ones8": ones.astype(NP_FP8),
            "biasv": np.full((P, 1), EXP_BIAS, np.float32),
        })
    return in_maps


def assemble(results):
    out = np.empty((B, T, C), np.float32)
    for core in range(8):
        b, role = divmod(core, 2)
        oTt = np.asarray(results[core]["outTb"]).astype(np.float32)  # [128,4,8,256]
        oT = oTt.transpose(2, 0, 1, 3).reshape(C, NS * P)
        rs = np.asarray(results[core]["rows"]).reshape(NS * P)
        for j in range(NS):
            sl = slice(j * P, (j + 1) * P)
            o = oT[:, sl].T / rs[sl][:, None]
            if j >= FP8_FROM:
                o = o * OUT_RESCALE
            r0 = 256 * j + P * role
            out[b, r0:r0 + P] = o
    return out


def kernel(x, Wq, Wk, Wv):
    nc = _get_nc()
    in_maps = make_in_maps(x, Wq, Wk, Wv)
    res = bass_utils.run_bass_kernel_spmd(nc, in_maps, core_ids=list(range(8)))
    return assemble(res.results)


def _install_trace_shim():
    """Provide antenv.axon_hooks (absent in this image) so trace=True works."""
    import sys
    import types
    if "antenv.axon_hooks" in sys.modules:
        return
    hook_box = [None]
    mod = types.ModuleType("antenv.axon_hooks")
    mod.set_axon_ntff_profile_hook = lambda h: hook_box.__setitem__(0, h)
    mod.get_axon_ntff_profile_hook = lambda: hook_box[0]
    import antenv
    sys.modules["antenv.axon_hooks"] = mod
    antenv.axon_hooks = mod
    try:
        from trn_agent_boot.trn_boot import _ntff_profile_via_ctypes
        mod.set_axon_ntff_profile_hook(
            _ntff_profile_via_ctypes("/opt/axon/libaxon_pjrt.so"))
    except Exception:
        pass


def run_traced(x, Wq, Wk, Wv):
    """Like kernel() but with NTFF tracing; returns (out, BassKernelResults)."""
    _install_trace_shim()
    nc = _get_nc()
    in_maps = make_in_maps(x, Wq, Wk, Wv)
    res = bass_utils.run_bass_kernel_spmd(
        nc, in_maps, core_ids=list(range(8)), trace=True,
        trace_cores=list(range(8)))
    return assemble(res.results), res


# revision 17
# speedup vs baseline: 1.2738x; 1.0394x over previous

"""Causal attention (no head split) on 8 trn2 NeuronCores — v2.

Math (per batch b), K/V never materialized:
    S   = x M x^T with M = Wq^T Wk precomputed on host
    out = (wei x) Wv^T; device computes H = x^T wei^T then O^T = Wv H.

v2 structure (vs v1):
  * strip-major schedule: 8 query strips of 128 rows per core; strip j
    attends key chunks 0..j (256 keys each). Exact causal tiling for both
    roles (role A takes even 128-row blocks, role B odd) — zero fully
    masked visits.
  * logit path bf16; strips >= FP8_FROM run P/xn/H-cast/Wv in fp8 e4m3
    with DoubleRow matmuls; earlier strips (small softmax rows) stay bf16.
  * H and rowsum accumulate in PSUM across a strip's chunks.
  * causal mask applied additively to S *before* exp (no fp8 inf*0).
  * all inputs pre-tiled on host to [128, X] contiguous layouts; one DMA
    per tensor half, staggered so the G-phase gate (wm, xq) lands first.
"""
import numpy as np
import ml_dtypes

import concourse.bass as bass
from concourse import bacc
import concourse.mybir as mybir
from concourse.tile import TileContext
from concourse import bass_utils

B, T, C = 4, 2048, 1024
P = 128
CS = C // P           # 8 contraction subtiles
NS = 8                # strips per core (128 queries each)
SCALE = 1.0 / np.sqrt(C)   # 1/32
EXP_BIAS = -2.5
FP8_FROM = 2          # strips >= this use fp8 path
H_SCALE = 0.25        # H -> fp8 cast scale
WV_SCALE = 64.0       # Wv fp8 scale
OUT_RESCALE = 1.0 / (H_SCALE * WV_SCALE)
MASK_NEG = -65536.0   # additive mask for non-causal logits

BF16 = mybir.dt.bfloat16
FP8 = mybir.dt.float8e4
F32 = mybir.dt.float32
DR = mybir.MatmulPerfMode.DoubleRow

NP_BF16 = ml_dtypes.bfloat16
NP_FP8 = ml_dtypes.float8_e4m3


def _tile128(a):
    """[(g*128), X] row-major -> [128, g, X] (partition-major pre-tiling)."""
    g = a.shape[0] // P
    return np.ascontiguousarray(a.reshape(g, P, -1).transpose(1, 0, 2))


def build():
    nc = bacc.Bacc(trn_type="TRN2", name="causal_attn_v2")
    xt_d = [nc.dram_tensor(f"xt{k}", [P, CS, 512], BF16, kind="ExternalInput")
            for k in range(4)]
    xqp_d = [nc.dram_tensor(f"xqp{k}", [P, CS, 256], BF16, kind="ExternalInput")
             for k in range(4)]
    wmp_d = [nc.dram_tensor(f"wmp{k}", [P, CS, 256], BF16, kind="ExternalInput")
             for k in range(4)]
    xnb_d = nc.dram_tensor("xnb", [P, 4, C], BF16, kind="ExternalInput")
    xn8_d = [nc.dram_tensor(f"xn8{k}", [P, 8, C], FP8, kind="ExternalInput")
             for k in range(2)]
    wvb_d = nc.dram_tensor("wvb", [P, CS, C], BF16, kind="ExternalInput")
    wv8_d = nc.dram_tensor("wv8", [P, CS, C], FP8, kind="ExternalInput")
    msk_d = nc.dram_tensor("msk", [P, 2, P], BF16, kind="ExternalInput")
    onesb_d = nc.dram_tensor("onesb", [P, 1], BF16, kind="ExternalInput")
    ones8_d = nc.dram_tensor("ones8", [P, 1], FP8, kind="ExternalInput")
    biasv_d = nc.dram_tensor("biasv", [P, 1], F32, kind="ExternalInput")
    outT_d = nc.dram_tensor("outTb", [P, 4, CS, 256], BF16, kind="ExternalOutput")
    rows_d = nc.dram_tensor("rows", [1, NS * P], F32, kind="ExternalOutput")
    rows_r = rows_d.rearrange("p (j t) -> p j t", j=NS)

    with TileContext(nc) as tc:
        with tc.tile_pool(name="keep", bufs=1) as keep, \
             tc.tile_pool(name="hrpool", bufs=2) as hrpool, \
             tc.tile_pool(name="opool", bufs=2) as opool, \
             tc.tile_pool(name="ppool", bufs=10) as ppool, \
             tc.tile_pool(name="psA", bufs=2, space="PSUM") as psA, \
             tc.tile_pool(name="psS", bufs=2, space="PSUM") as psS, \
             tc.tile_pool(name="psH", bufs=1, space="PSUM") as psH, \
             tc.tile_pool(name="psR", bufs=1, space="PSUM") as psR, \
             tc.tile_pool(name="psR2", bufs=1, space="PSUM") as psR2:

            # ---------------- resident tiles ----------------
            xt = [keep.tile([P, CS, 512], BF16, name=f"xt{k}", tag=f"xt{k}") for k in range(4)]
            xqp = [keep.tile([P, CS, 256], BF16, name=f"xqp{k}", tag=f"xqp{k}")
                   for k in range(4)]
            wmp = [keep.tile([P, CS, 256], BF16, name=f"wmp{k}", tag=f"wmp{k}")
                   for k in range(4)]
            gT = keep.tile([P, CS, NS * P], BF16, tag="gT")     # 16KB
            xnb_t = keep.tile([P, 4, C], BF16, tag="xnb")       # 8KB
            xn8_t = [keep.tile([P, 8, C], FP8, name=f"xn8{k}", tag=f"xn8{k}") for k in range(2)]
            wvb_t = keep.tile([P, CS, C], BF16, tag="wvb")      # 16KB
            wv8_t = keep.tile([P, CS, C], FP8, tag="wv8")       # 8KB
            mt = keep.tile([P, 2, P], BF16, tag="mt")
            ob = keep.tile([P, 1], BF16, tag="ob")
            o8 = keep.tile([P, 1], FP8, tag="o8")
            bv = keep.tile([P, 1], F32, tag="bv")
            rsum = keep.tile([1, NS, P], F32, tag="rsum")

            warm = keep.tile([P, 256], BF16, tag="warm")
            # -------- PE prewarm: dummy matmuls while DMAs land ---------
            # HAM un-throttles 1.2 -> 2.4 GHz after ~3.4us of sustained PE
            # activity; warm it up before the first real chain.
            nc.vector.memset(warm[:], 0.0)
            pw = None
            for w in range(32):
                pw = psA.tile([P, 256], F32, tag="pq")
                nc.tensor.matmul(pw[:], warm[:, 0:P], warm[:],
                                 start=True, stop=True)
            nc.vector.tensor_copy(warm[:], pw[:])

            # -------- DMAs: whole-tensor (contiguous), 3 queues ---------
            # G-gate pieces spread across queues in wavefront-need order.
            nc.sync.dma_start(xqp[1][:], xqp_d[1][:])
            nc.sync.dma_start(wmp[1][:], wmp_d[1][:])
            nc.sync.dma_start(xqp[3][:], xqp_d[3][:])
            nc.sync.dma_start(xt[2][:], xt_d[2][:])
            nc.sync.dma_start(xt[3][:], xt_d[3][:])
            nc.scalar.dma_start(xqp[0][:], xqp_d[0][:])
            nc.scalar.dma_start(wmp[2][:], wmp_d[2][:])
            nc.scalar.dma_start(xqp[2][:], xqp_d[2][:])
            nc.scalar.dma_start(wvb_t[:], wvb_d[:])
            nc.scalar.dma_start(wv8_t[:], wv8_d[:])
            nc.gpsimd.dma_start(wmp[0][:], wmp_d[0][:])
            nc.gpsimd.dma_start(wmp[3][:], wmp_d[3][:])
            nc.gpsimd.dma_start(mt[:], msk_d[:])
            nc.gpsimd.dma_start(ob[:], onesb_d[:])
            nc.gpsimd.dma_start(o8[:], ones8_d[:])
            nc.gpsimd.dma_start(bv[:], biasv_d[:])
            nc.gpsimd.dma_start(xnb_t[:], xnb_d[:])
            nc.gpsimd.dma_start(xt[0][:], xt_d[0][:])
            nc.gpsimd.dma_start(xt[1][:], xt_d[1][:])
            nc.gpsimd.dma_start(xn8_t[0][:], xn8_d[0][:])
            nc.gpsimd.dma_start(xn8_t[1][:], xn8_d[1][:])

            # ---------------- Phase G: gT = M^T xq^T (bf16) -------------
            # G chains in wavefront order (2q + ds) so the wm and xq
            # pieces are consumed as their transfers land.
            chains = sorted(((q, ds) for q in range(4) for ds in range(CS)),
                            key=lambda t: (2 * t[0] + t[1], t[0]))
            for q, ds in chains:
                qs = slice(256 * q, 256 * (q + 1))
                pq = psA.tile([P, 256], F32, tag="pq")
                for cs in range(CS):
                    nc.tensor.matmul(
                        pq[:], wmp[ds // 2][:, cs, (ds % 2) * P:(ds % 2) * P + P],
                        xqp[q][:, cs, :],
                        start=(cs == 0), stop=(cs == CS - 1))
                nc.vector.tensor_copy(gT[:, ds, qs], pq[:])

            # ---------------- strip-pair attention ----------------------
            # Strips 2k and 2k+1 share key chunks 0..2k: process them as a
            # 256-column pair (S/exp at 256 cols), then one 128-col visit
            # for strip 2k+1's diagonal chunk. Zero masked-visit waste.
            pending_proj = None
            for k in range(4):
                j, j2 = 2 * k, 2 * k + 1
                f8 = j >= FP8_FROM
                one = o8 if f8 else ob
                pdt = FP8 if f8 else BF16
                cols = slice(j * P, j * P + 256)
                rw = psR.tile([1, 256], F32, tag="rw")
                pts = []

                def emit_rw(c):
                    for ss in range(2):
                        nc.tensor.matmul(rw[:], one[:], pts[c][:, ss],
                                         start=(c == 0 and ss == 0),
                                         stop=(c == j and ss == 1))

                for c in range(j + 1):
                    st = psS.tile([P, 2, 256], F32, tag="st")
                    for ss in range(2):
                        for cs in range(CS):
                            off = 256 * (c % 2) + P * ss
                            nc.tensor.matmul(
                                st[:, ss], xt[c // 2][:, cs, off:off + P],
                                gT[:, cs, cols],
                                start=(cs == 0), stop=(cs == CS - 1))
                    if c == 0 and pending_proj is not None:
                        pending_proj()
                        pending_proj = None
                    # rowsum for the previous visit: its exp has had a full
                    # S-chain of time, so this never stalls the PE queue.
                    if c >= 1:
                        emit_rw(c - 1)
                    if c == j:  # diagonal of strip 2k: mask its 128 cols
                        nc.vector.tensor_add(st[:, :, 0:P], st[:, :, 0:P], mt[:])
                    pT = ppool.tile([P, 2, 256], pdt, tag="pT")
                    nc.scalar.activation(
                        pT[:], st[:], mybir.ActivationFunctionType.Exp,
                        scale=float(SCALE), bias=bv[:])
                    pts.append(pT)
                # diagonal visit for strip 2k+1 (chunk j+1, 128 cols):
                # emit its S chain first so exp(j) completes behind it.
                c = j + 1
                st2 = psS.tile([P, 2, P], F32, tag="st")
                for ss in range(2):
                    for cs in range(CS):
                        off = 256 * (c % 2) + P * ss
                        nc.tensor.matmul(
                            st2[:, ss], xt[c // 2][:, cs, off:off + P],
                            gT[:, cs, j2 * P:(j2 + 1) * P],
                            start=(cs == 0), stop=(cs == CS - 1))
                emit_rw(j)
                nc.vector.tensor_add(st2[:], st2[:], mt[:])
                pT2 = ppool.tile([P, 2, P], pdt, tag="pT")
                nc.scalar.activation(
                    pT2[:], st2[:], mybir.ActivationFunctionType.Exp,
                    scale=float(SCALE), bias=bv[:])
                pts.append(pT2)

                # H burst for strip 2k (left half of each pair pT)
                po = psH.tile([P, CS, P], F32, tag="po")
                for cs in range(CS):
                    for c in range(j + 1):
                        first, last = (c == 0), (c == j)
                        if f8:
                            hh = (2 * c) % 8
                            nc.tensor.matmul(
                                po[:, cs],
                                xn8_t[c // 4][:, hh:hh + 2, cs * P:(cs + 1) * P],
                                pts[c][:, 0:2, 0:P],
                                start=first, stop=last, perf_mode=DR)
                        else:
                            for ss in range(2):
                                nc.tensor.matmul(
                                    po[:, cs],
                                    xnb_t[:, 2 * c + ss, cs * P:(cs + 1) * P],
                                    pts[c][:, ss, 0:P],
                                    start=(first and ss == 0),
                                    stop=(last and ss == 1))
                rw2 = psR2.tile([1, P], F32, tag="rw2")
                for ss in range(2):
                    nc.tensor.matmul(rw2[:], one[:], pts[j + 1][:, ss],
                                     start=(ss == 0), stop=(ss == 1))
                nc.vector.tensor_copy(rsum[:, j], rw[:, 0:P])
                hr = hrpool.tile([P, CS, 256], pdt, tag="hr")
                nc.scalar.activation(
                    hr[:, :, 0:P], po[:],
                    mybir.ActivationFunctionType.Copy,
                    scale=float(H_SCALE) if f8 else 1.0)

                # H burst for strip 2k+1 (right half + diagonal pT2)
                po2 = psH.tile([P, CS, P], F32, tag="po")
                for cs in range(CS):
                    for c in range(j + 2):
                        first, last = (c == 0), (c == j + 1)
                        psl = (pts[c][:, 0:2, P:256] if c <= j
                               else pts[c][:, 0:2, 0:P])
                        if f8:
                            hh = (2 * c) % 8
                            nc.tensor.matmul(
                                po2[:, cs],
                                xn8_t[c // 4][:, hh:hh + 2, cs * P:(cs + 1) * P],
                                psl, start=first, stop=last, perf_mode=DR)
                        else:
                            for ss in range(2):
                                nc.tensor.matmul(
                                    po2[:, cs],
                                    xnb_t[:, 2 * c + ss, cs * P:(cs + 1) * P],
                                    psl[:, ss], start=(first and ss == 0),
                                    stop=(last and ss == 1))
                nc.vector.tensor_copy(rsum[:, j2], rw[:, P:256])
                nc.vector.tensor_add(rsum[:, j2], rsum[:, j2], rw2[:])
                if k == 3:
                    nc.sync.dma_start(rows_r[:], rsum[:])
                nc.scalar.activation(
                    hr[:, :, P:256], po2[:],
                    mybir.ActivationFunctionType.Copy,
                    scale=float(H_SCALE) if f8 else 1.0)

                # pair projection (emitted during the next pair's first S
                # chain so the hr cast never stalls the PE queue)
                def make_proj(k, f8, hr):
                    def do_proj():
                        ost = opool.tile([P, CS, 256], BF16, tag="ost")
                        for ds in range(CS):
                            pf = psA.tile([P, 256], F32, tag="pq")
                            if f8:
                                for h in range(4):
                                    nc.tensor.matmul(
                                        pf[:],
                                        wv8_t[:, 2 * h:2 * h + 2, ds * P:(ds + 1) * P],
                                        hr[:, 2 * h:2 * h + 2, :],
                                        start=(h == 0), stop=(h == 3),
                                        perf_mode=DR)
                            else:
                                for cs in range(CS):
                                    nc.tensor.matmul(
                                        pf[:], wvb_t[:, cs, ds * P:(ds + 1) * P],
                                        hr[:, cs, :],
                                        start=(cs == 0), stop=(cs == CS - 1))
                            nc.vector.tensor_copy(ost[:, ds], pf[:])
                        nc.gpsimd.dma_start(outT_d[:, k], ost[:])
                    return do_proj

                pending_proj = make_proj(k, f8, hr)
            pending_proj()

    nc.compile()
    return nc


_NC = None


def _get_nc():
    global _NC
    if _NC is None:
        _NC = build()
    return _NC


def make_in_maps(x, Wq, Wk, Wv):
    x = np.asarray(x, dtype=np.float32)
    wq64 = np.asarray(Wq, np.float64)
    wk64 = np.asarray(Wk, np.float64)
    wm = (wq64.T @ wk64).astype(np.float32)
    wm_t = _tile128(wm.astype(NP_BF16))
    wvT = np.ascontiguousarray(np.asarray(Wv, np.float32).T)
    wvb_t = _tile128(wvT.astype(NP_BF16))
    wv8_t = _tile128((wvT * WV_SCALE).astype(NP_FP8))
    ones = np.ones((P, 1), np.float32)

    # additive masks [p(key sub), ss, t]: 0 = keep, MASK_NEG = drop.
    p_idx = np.arange(P)[:, None]
    t_idx = np.arange(P)[None, :]
    tri = np.where(p_idx <= t_idx, 0.0, MASK_NEG).astype(np.float32)
    zero = np.full((P, P), MASK_NEG, np.float32)
    full = np.zeros((P, P), np.float32)
    mask_A = np.stack([tri, zero], axis=1)
    mask_B = np.stack([full, tri], axis=1)

    in_maps = []
    for core in range(8):
        b, role = divmod(core, 2)
        xb = x[b]
        xTb = np.ascontiguousarray(xb.T).astype(NP_BF16)
        cols = np.concatenate(
            [np.arange(256 * j + P * role, 256 * j + P * role + P)
             for j in range(NS)])
        xt_t = _tile128(xTb)                     # [128, 8, 2048]
        xq_t = _tile128(np.ascontiguousarray(xTb[:, cols]))
        xn8_t = _tile128(xb.astype(NP_FP8))      # [128, 16, 1024]
        in_maps.append({
            **{f"xt{k}": np.ascontiguousarray(xt_t[:, :, 512 * k:512 * (k + 1)])
               for k in range(4)},
            **{f"xqp{k}": np.ascontiguousarray(xq_t[:, :, 256 * k:256 * (k + 1)])
               for k in range(4)},
            **{f"wmp{k}": np.ascontiguousarray(wm_t[:, :, 256 * k:256 * (k + 1)])
               for k in range(4)},
            "xnb": _tile128(xb[0:512].astype(NP_BF16)),
            "xn80": np.ascontiguousarray(xn8_t[:, 0:8]),
            "xn81": np.ascontiguousarray(xn8_t[:, 8:16]),
            "wvb": wvb_t,
            "wv8": wv8_t,
            "msk": (mask_A if role == 0 else mask_B).astype(NP_BF16),
            "onesb": ones.astype(NP_BF16),
            "ones8": ones.astype(NP_FP8),
            "biasv": np.full((P, 1), EXP_BIAS, np.float32),
        })
    return in_maps


def assemble(results):
    out = np.empty((B, T, C), np.float32)
    for core in range(8):
        b, role = divmod(core, 2)
        oTt = np.asarray(results[core]["outTb"]).astype(np.float32)  # [128,4,8,256]
        oT = oTt.transpose(2, 0, 1, 3).reshape(C, NS * P)
        rs = np.asarray(results[core]["rows"]).reshape(NS * P)
        for j in range(NS):
            sl = slice(j * P, (j + 1) * P)
            o = oT[:, sl].T / rs[sl][:, None]
            if j >= FP8_FROM:
                o = o * OUT_RESCALE
            r0 = 256 * j + P * role
            out[b, r0:r0 + P] = o
    return out


def kernel(x, Wq, Wk, Wv):
    nc = _get_nc()
    in_maps = make_in_maps(x, Wq, Wk, Wv)
    res = bass_utils.run_bass_kernel_spmd(nc, in_maps, core_ids=list(range(8)))
    return assemble(res.results)


def _install_trace_shim():
    """Provide antenv.axon_hooks (absent in this image) so trace=True works."""
    import sys
    import types
    if "antenv.axon_hooks" in sys.modules:
        return
    hook_box = [None]
    mod = types.ModuleType("antenv.axon_hooks")
    mod.set_axon_ntff_profile_hook = lambda h: hook_box.__setitem__(0, h)
    mod.get_axon_ntff_profile_hook = lambda: hook_box[0]
    import antenv
    sys.modules["antenv.axon_hooks"] = mod
    antenv.axon_hooks = mod
    try:
        from trn_agent_boot.trn_boot import _ntff_profile_via_ctypes
        mod.set_axon_ntff_profile_hook(
            _ntff_profile_via_ctypes("/opt/axon/libaxon_pjrt.so"))
    except Exception:
        pass


def run_traced(x, Wq, Wk, Wv):
    """Like kernel() but with NTFF tracing; returns (out, BassKernelResults)."""
    _install_trace_shim()
    nc = _get_nc()
    in_maps = make_in_maps(x, Wq, Wk, Wv)
    res = bass_utils.run_bass_kernel_spmd(
        nc, in_maps, core_ids=list(range(8)), trace=True,
        trace_cores=list(range(8)))
    return assemble(res.results), res
